# revision 6
# baseline (speedup 1.0000x reference)
"""Trainium2 Bass kernel for EntityMarker segment-reduce (span means).

Problem: sequence_output [128, 2048, 768] f32, entity_positions [128, 4] int.
For each batch b, compute the mean of sequence_output[b, s:e+1, :] for the
head span (cols 0,1) and tail span (cols 2,3), clamped like the reference.
Output: (head [128, 768], tail [128, 768]) f32.

v3 strategy (host-packed fp16 + direct HWDGE DMA):
  - HBM-bandwidth bound: only the union of the two spans (~26% of rows)
    must be read. The host splits each batch's union into "zones" of
    constant (head, tail) membership, chops zones into K=8-row windows
    (zero-padded), and packs the window rows CONTIGUOUSLY per core in
    fp16 (halves device HBM traffic; fp16 error ~3e-4 vs 2e-2 gate).
  - Device: chunk DMAs pull m*128 windows with 24KB-per-partition
    descriptors (m=2). A ramp schedule (32, 96, then 256-window chunks)
    fills the pipeline early. Per 128-window sub-chunk, a 2-level DVE
    pairwise tree reduces 8 rows -> 2 blocks, and TWO fp16 matmul pairs
    with the same 0/1 slot-selection lhsT accumulate both blocks into
    PSUM (the PE absorbs the last tree level; 1/span_len scaling is
    applied on host in f32). Pool engine is NOT used for the tree: DVE
    and Pool contend on SBUF and both slow down ~2-4x.
  - Output slots map (core, slot) -> (batch, head/tail); zones split
    across cores yield partial sums the host adds before scaling.
  - The program is uniform across cores (SPMD).
"""

import os

import numpy as np

_B, _L, _H = 128, 2048, 768
_NCORES = 8
_K = 8   # rows per window
_GBUFS = int(os.environ.get("KERNEL_GBUFS", "4"))

_prog_cache = {}


def _schedule(per_core):
    """Chunk schedule: list of (window offset, partitions, m windows each).

    Small leading chunks hide the first-transfer latency; then paired
    chunks (m=2) give 24KB descriptors.
    """
    sch = []
    off = 0
    rem = per_core
    for first in (32, 96):
        take = min(first, rem)
        if take:
            sch.append((off, take, 1))
            off += take
            rem -= take
    while rem >= 256:
        sch.append((off, 128, 2))
        off += 256
        rem -= 256
    if rem > 128:
        p = (rem + 1) // 2
        sch.append((off, p, 2))
        off += 2 * p
        rem = 0
    elif rem:
        sch.append((off, rem, 1))
        off += rem
        rem = 0
    return sch, off  # off >= per_core: padded per-core window count


def _build_program(sched, tot_win, nslot):
    import concourse.mybir as mybir
    from concourse import bacc, tile

    f16 = mybir.dt.float16
    f32 = mybir.dt.float32
    h = _H
    kh = _K * _H
    n_sub = sum(m for _, _, m in sched)

    nc = bacc.Bacc(None, target_bir_lowering=False)
    x = nc.declare_dram_parameter("x", [tot_win, kh], f16, isOutput=False)
    w = nc.declare_dram_parameter("w", [128, n_sub * nslot], f16,
                                  isOutput=False)
    out = nc.declare_dram_parameter("out", [nslot, _H], f32, isOutput=True)

    with tile.TileContext(nc) as tc:
        with (
            tc.tile_pool(name="const", bufs=1) as cpool,
            tc.tile_pool(name="gather", bufs=_GBUFS) as gpool,
            tc.tile_pool(name="tree", bufs=3) as tpool,
            tc.tile_pool(name="psum", bufs=1, space="PSUM") as ppool,
        ):
            # w load on the scalar HWDGE ring so the sync ring's first
            # chunk DMA issues immediately after the preamble
            w_t = cpool.tile([128, n_sub * nslot], f16)
            nc.scalar.dma_start(out=w_t[:], in_=w[:])

            ps_a = ppool.tile([nslot, 512], f32)
            ps_b = ppool.tile([nslot, 256], f32)

            n_mm = 2 * n_sub  # two PSUM-accumulating mm pairs per sub-chunk
            issued = [0]
            sub = [0]

            def mm_pair(p, lhsT, rhs):
                st = issued[0] == 0
                sp = issued[0] == n_mm - 1
                issued[0] += 1
                nc.tensor.matmul(ps_a[:], lhsT, rhs[:p, 0:512],
                                 start=st, stop=sp)
                nc.tensor.matmul(ps_b[:], lhsT, rhs[:p, 512:h],
                                 start=st, stop=sp)

            for (off, p, m) in sched:
                g = gpool.tile([128, m * kh], f16, tag="g")
                src = x[off:off + p * m, :]
                if m > 1:
                    src = src.rearrange("(p m) d -> p (m d)", m=m)
                # SWDGE: measured ~27 B/ns per SDMA engine vs ~20 for
                # HWDGE on identical 24KB descriptors
                nc.gpsimd.dma_start(out=g[:p], in_=src)
                for j in range(m):
                    gs = g[:p, j * kh:(j + 1) * kh]
                    # level 1: 8 rows -> 4 blocks
                    a1 = tpool.tile([128, 4 * h], f16, tag="a1")
                    s1 = gs.rearrange("p (k two h) -> p k two h", two=2, h=h)
                    nc.vector.tensor_add(
                        a1[:p].rearrange("p (k h) -> p k h", h=h),
                        s1[:, :, 0, :], s1[:, :, 1, :])
                    # level 2: 4 blocks -> 2 blocks
                    a2 = tpool.tile([128, 2 * h], f16, tag="a2")
                    s2 = a1[:p].rearrange("p (k two h) -> p k two h",
                                          two=2, h=h)
                    nc.vector.tensor_add(
                        a2[:p].rearrange("p (k h) -> p k h", h=h),
                        s2[:, :, 0, :], s2[:, :, 1, :])
                    # PE absorbs level 3: both blocks hit the same lhsT
                    lhsT = w_t[:p, sub[0] * nslot:(sub[0] + 1) * nslot]
                    sub[0] += 1
                    mm_pair(p, lhsT, a2[:p, 0:h])
                    mm_pair(p, lhsT, a2[:p, h:2 * h])

            o_t = cpool.tile([nslot, _H], f32)
            nc.vector.tensor_copy(o_t[:, 0:512], ps_a[:])
            nc.scalar.copy(o_t[:, 512:_H], ps_b[:])
            nc.scalar.dma_start(out=out[:], in_=o_t[:])
    nc.compile()
    return nc


def _spans(entity_positions):
    ep = np.asarray(entity_positions).astype(np.int64)
    hs = np.clip(ep[:, 0], 0, _L - 1)
    he = np.maximum(hs, np.minimum(ep[:, 1], _L - 1))
    ts = np.clip(ep[:, 2], 0, _L - 1)
    te = np.maximum(ts, np.minimum(ep[:, 3], _L - 1))
    return hs, he, ts, te


def _plan(entity_positions):
    """Zones -> K-row windows -> row-balanced core shards."""
    hs, he, ts, te = _spans(entity_positions)

    # zones of constant (head, tail) membership, per batch
    zones = []  # (b, s, e, inH, inT)
    for b in range(_B):
        cuts = sorted({int(hs[b]), int(he[b]) + 1, int(ts[b]), int(te[b]) + 1})
        for a, c in zip(cuts[:-1], cuts[1:]):
            iH = hs[b] <= a <= he[b]
            iT = ts[b] <= a <= te[b]
            if iH or iT:
                zones.append((b, a, c - 1, iH, iT))

    # windows: K consecutive rows of one zone (last window zero-padded)
    win_meta = []   # (b, iH, iT)
    win_rows = []   # [K] flat row indices, pad = B*L (points at zero row)
    pad_row = _B * _L
    for (b, s, e, iH, iT) in zones:
        base = b * _L
        r = s
        while r <= e:
            k = min(_K, e - r + 1)
            rows = np.full(_K, pad_row, np.int64)
            rows[:k] = base + np.arange(r, r + k)
            win_rows.append(rows)
            win_meta.append((b, iH, iT))
            r += k

    n_win = len(win_meta)
    per_core = (n_win + _NCORES - 1) // _NCORES
    sched, tot_win = _schedule(per_core)

    # pad the global list so every core has exactly tot_win windows
    pad_meta = (None, False, False)
    need = tot_win * _NCORES
    grid_meta = []
    grid_rows = np.full((need, _K), pad_row, np.int64)
    for c in range(_NCORES):
        lo = c * per_core
        seg = win_meta[lo:lo + per_core]
        grid_meta.extend(seg + [pad_meta] * (tot_win - len(seg)))
        rows = win_rows[lo:lo + per_core]
        if rows:
            grid_rows[c * tot_win:c * tot_win + len(rows)] = np.asarray(rows)

    # per-core slot assignment
    slot_maps = []
    core_slots = []
    for c in range(_NCORES):
        seg = grid_meta[c * tot_win:(c + 1) * tot_win]
        smap = {}
        for (b, iH, iT) in seg:
            if b is None:
                continue
            if iH and (b, 'h') not in smap:
                smap[(b, 'h')] = len(smap)
            if iT and (b, 't') not in smap:
                smap[(b, 't')] = len(smap)
        core_slots.append(smap)
        slot_maps.append([k for k, _ in sorted(smap.items(),
                                               key=lambda kv: kv[1])])
    nslot = max(1, max(len(s) for s in core_slots))
    assert nslot <= 128, f"slot overflow: {nslot}"

    # weight matrices: sub-chunk sc, partition q -> window off + m*q + j
    n_sub = sum(m for _, _, m in sched)
    w_mats = []
    for c in range(_NCORES):
        seg = grid_meta[c * tot_win:(c + 1) * tot_win]
        smap = core_slots[c]
        wm = np.zeros((128, n_sub * nslot), np.float16)
        sc = 0
        for (off, p, m) in sched:
            for j in range(m):
                for q in range(p):
                    b, iH, iT = seg[off + m * q + j]
                    if b is None:
                        continue
                    if iH:
                        wm[q, sc * nslot + smap[(b, 'h')]] = 1.0
                    if iT:
                        wm[q, sc * nslot + smap[(b, 't')]] = 1.0
                sc += 1
        w_mats.append(wm)

    return grid_rows, w_mats, slot_maps, sched, tot_win, nslot


def _run(sequence_output, entity_positions, trace=False, trace_cores=None):
    from concourse.bass_utils import run_bass_kernel_spmd

    x = np.asarray(sequence_output, dtype=np.float32).reshape(_B * _L, _H)
    grid_rows, w_mats, slot_maps, sched, tot_win, nslot = _plan(
        entity_positions)

    key = (tuple(sched), tot_win, nslot)
    if key not in _prog_cache:
        _prog_cache[key] = _build_program(sched, tot_win, nslot)
    nc = _prog_cache[key]

    # fp16 copy with one zero row appended for window padding
    x16 = np.empty((_B * _L + 1, _H), np.float16)
    x16[:_B * _L] = x
    x16[_B * _L] = 0
    in_maps = []
    for c in range(_NCORES):
        rows = grid_rows[c * tot_win:(c + 1) * tot_win].reshape(-1)
        xc = x16[rows].reshape(tot_win, _K * _H)
        in_maps.append({"x": xc, "w": w_mats[c]})

    res = run_bass_kernel_spmd(
        nc, in_maps, list(range(_NCORES)), trace=trace,
        trace_cores=trace_cores,
    )

    hs, he, ts, te = _spans(entity_positions)
    head = np.zeros((_B, _H), np.float32)
    tail = np.zeros((_B, _H), np.float32)
    for c in range(_NCORES):
        o = np.asarray(res.results[c]["out"], np.float32)
        for s, (b, role) in enumerate(slot_maps[c]):
            if role == 'h':
                head[b] += o[s]
            else:
                tail[b] += o[s]
    head /= (he - hs + 1).astype(np.float32)[:, None]
    tail /= (te - ts + 1).astype(np.float32)[:, None]
    return (head, tail), res


def kernel(sequence_output, entity_positions):
    (head, tail), _ = _run(sequence_output, entity_positions)
    return head, tail
